# revision 13
# baseline (speedup 1.0000x reference)
"""CoAttLayer Trainium2 kernel — pure data-parallel over batch on 8 NeuronCores.

Reference computation (per batch element b, T=1024, N=512, D=64, K=80):
  L  = tanh(R @ Wl @ P^T)                    (T, N)
  Hp = tanh(Wp @ P^T + (Wr @ R^T) @ L)       (K, N)
  Hr = tanh(Wr @ R^T + (Wp @ P^T) @ L^T)     (K, T)
  Ap = softmax(whp @ Hp), Ar = softmax(whr @ Hr)
  out[b] = concat(P^T @ Ap, R^T @ Ar)        (2D,)

Reassociated into D-sized contractions:
  Hp = [Wp | Wr] @ [P^T ; X]   with X = R^T @ L    (D, N)
  Hr = [Wr | Wp] @ [R^T ; Y]   with Y = P^T @ L^T  (D, T)

Design notes (from trace analysis):
 - The PE HAM clock governor only counts real matmul activity; transpose-mode
   instructions poison it back to 1.2 GHz. So the batch loop contains ZERO PE
   transposes: all static transposed layouts (R^T, P^T, weight stacks) are
   prepared on the HOST, and the data-dependent L^T is produced by bouncing
   L through DRAM and reading it back through the DMA xbar transpose engine
   (~180 GB/s, fully off the compute engines).
 - All matmul operands are bf16 (fp32 PSUM accumulate); tanh lives on the
   Scalar engine with 1024-wide evacuations; PSUM evacuations go to DVE.
 - Softmax is batched across the 8 local batch elements on partitions.
"""

import numpy as np

import concourse.bass as bass
import concourse.bacc as bacc
import concourse.mybir as mybir
import concourse.tile as tile
from concourse.bass_utils import run_bass_kernel_spmd

F32 = mybir.dt.float32
BF16 = mybir.dt.bfloat16
AF = mybir.ActivationFunctionType

B_LOC = 8      # batch elements per core
T, N, D, K = 1024, 512, 64, 80
TI = T // 128  # 8 t-tiles
NI = N // 128  # 4 n-tiles
NCORES = 8


def build_kernel():
    nc = bacc.Bacc("TRN2", debug=False, target_bir_lowering=False)

    ins = {}
    for name, shape, dt in [
        ("review_bf", [B_LOC, T, D], BF16),
        ("review_t", [B_LOC, D, T], BF16),
        ("post_bf", [B_LOC, N, D], BF16),
        ("post_t", [B_LOC, D, N], BF16),
        ("wl_b", [D, D], BF16),
        ("wt_hp", [2 * D, K], BF16),
        ("wt_hr", [2 * D, K], BF16),
        ("whp_c", [K, 1], BF16),
        ("whr_c", [K, 1], BF16),
        ("ident", [128, 128], F32),
    ]:
        ins[name] = nc.declare_dram_parameter(name, shape, dt, isOutput=False)
    out_e = nc.declare_dram_parameter("out", [B_LOC, 2 * D], F32, isOutput=True)

    with tile.TileContext(nc) as tc:
        _body(nc, tc, ins, out_e)

    nc.compile()
    return nc


def _body(nc, tc, ins, out_e):
    from contextlib import ExitStack

    ctx = ExitStack()
    cpool = ctx.enter_context(tc.tile_pool(name="const", bufs=1))
    inpool = ctx.enter_context(tc.tile_pool(name="inputs", bufs=1))
    wk = ctx.enter_context(tc.tile_pool(name="work", bufs=2))
    dpool = ctx.enter_context(tc.tile_pool(name="dram", bufs=2, space="DRAM"))
    ps_mm = ctx.enter_context(tc.tile_pool(name="ps_mm", bufs=3, space="PSUM"))
    ps_acc = ctx.enter_context(tc.tile_pool(name="ps_acc", bufs=2, space="PSUM"))

    # ---------------- constants (all pre-transposed on host) ----------------
    ident_f = cpool.tile([128, 128], F32)
    nc.sync.dma_start(out=ident_f[:], in_=ins["ident"].ap())
    wl_b = cpool.tile([D, D], BF16)
    nc.sync.dma_start(out=wl_b[:], in_=ins["wl_b"].ap())
    wt_hp = cpool.tile([2 * D, K], BF16)
    nc.sync.dma_start(out=wt_hp[:], in_=ins["wt_hp"].ap())
    wt_hr = cpool.tile([2 * D, K], BF16)
    nc.sync.dma_start(out=wt_hr[:], in_=ins["wt_hr"].ap())
    whp_b = cpool.tile([K, 1], BF16)
    nc.sync.dma_start(out=whp_b[:], in_=ins["whp_c"].ap())
    whr_b = cpool.tile([K, 1], BF16)
    nc.sync.dma_start(out=whr_b[:], in_=ins["whr_c"].ap())

    # Persistent bf16 inputs (matmul operands + pooling-phase lhsT)
    r_ball = inpool.tile([128, B_LOC, TI, D], BF16)
    p_ball = inpool.tile([128, B_LOC, NI, D], BF16)

    # Per-batch logits, transposed layout: cols 0:4 ap n-tiles, 4:12 ar t-tiles
    lgt_all = inpool.tile([128, 12, B_LOC], F32)

    # ---------------- per-batch main phase ----------------
    for b in range(B_LOC):
        nc.sync.dma_start(
            out=r_ball[:, b],
            in_=ins["review_bf"].ap()[b].rearrange("(i p) d -> p i d", p=128),
        )
        nc.sync.dma_start(
            out=p_ball[:, b],
            in_=ins["post_bf"].ap()[b].rearrange("(j p) d -> p j d", p=128),
        )
        # [R^T ; Y] and [P^T ; X] stacks; the transposed halves straight from DRAM
        hr_in = wk.tile([128, T], BF16, tag="hr_in")
        nc.sync.dma_start(out=hr_in[0:D, :], in_=ins["review_t"].ap()[b])
        hp_in = wk.tile([128, N], BF16, tag="hp_in")
        nc.sync.dma_start(out=hp_in[0:D, :], in_=ins["post_t"].ap()[b])

        # RlT[d', t] = sum_d Wl[d, d'] * Rt[d, t]   (D, T)
        rlt = wk.tile([D, T], BF16, tag="rlt")
        for c in range(2):
            rlt_ps = ps_acc.tile([D, 512], F32, tag="acc")
            nc.tensor.matmul(rlt_ps[:], wl_b[:], hr_in[0:D, c * 512:(c + 1) * 512])
            nc.vector.tensor_copy(rlt[:, c * 512:(c + 1) * 512], rlt_ps[:])

        # L tiles (pairs share a 2-bank PSUM tile -> one wide tanh evacuation),
        # X accumulation woven in to keep the PE stream dense.
        l_sb = wk.tile([128, TI, N], BF16, tag="l_sb")
        l_dram = dpool.tile([T, N], BF16, tag="l_dram")
        l_dram_v = l_dram.rearrange("(i p) n -> p i n", p=128)
        xps = ps_acc.tile([D, N], F32, tag="acc")
        lps = {}

        def emit_l_pair(k):
            lps[k] = ps_mm.tile([128, 2, N], F32, tag="mm", name=f"lps{k}")
            for h in range(2):
                i = 2 * k + h
                nc.tensor.matmul(
                    lps[k][:, h], rlt[:, i * 128:(i + 1) * 128], hp_in[0:D, :]
                )

        def emit_l_evac(k):
            nc.scalar.activation(l_sb[:, 2 * k:2 * k + 2, :], lps[k][:], AF.Tanh)
            nc.sync.dma_start(
                out=l_dram_v[:, 2 * k:2 * k + 2, :], in_=l_sb[:, 2 * k:2 * k + 2, :]
            )

        emit_l_pair(0)
        emit_l_pair(1)
        emit_l_pair(2)
        emit_l_evac(0)
        for i in range(TI):
            # X = R^T @ L  (D, N) accumulated over t-tiles -> HpIn[64:128]
            nc.tensor.matmul(
                xps[:], r_ball[:, b, i], l_sb[:, i], start=(i == 0), stop=(i == TI - 1)
            )
            if i % 2 == 1:
                k = i // 2
                if k + 3 < TI // 2:
                    emit_l_pair(k + 3)
                if k + 1 < TI // 2:
                    emit_l_evac(k + 1)
        nc.vector.tensor_copy(hp_in[D:128, :], xps[:])

        # L^T via DMA xbar transpose from the DRAM bounce of L
        lt_sb = wk.tile([128, NI, T], BF16, tag="lt_sb")
        for j in range(NI):
            nc.sync.dma_start_transpose(
                out=lt_sb[:, j], in_=l_dram[:, j * 128:(j + 1) * 128]
            )

        # Hp = tanh(WT_hp^T @ [Pt ; X])  (K, N)
        hp_bf = wk.tile([K, N], BF16, tag="hp_bf")
        hps = ps_acc.tile([K, N], F32, tag="acc")
        nc.tensor.matmul(hps[:], wt_hp[:], hp_in[:])
        nc.scalar.activation(hp_bf[:], hps[:], AF.Tanh)

        # Y = P^T @ L^T  (D, T) accumulated over n-tiles -> HrIn[64:128]
        yps = [ps_acc.tile([D, 512], F32, tag="acc", name=f"yps{c}") for c in range(2)]
        for j in range(NI):
            for c in range(2):
                nc.tensor.matmul(
                    yps[c][:], p_ball[:, b, j], lt_sb[:, j, c * 512:(c + 1) * 512],
                    start=(j == 0), stop=(j == NI - 1),
                )
        for c in range(2):
            nc.vector.tensor_copy(hr_in[D:128, c * 512:(c + 1) * 512], yps[c][:])

        # Hr = tanh(WT_hr^T @ [Rt ; Y])  (K, T)
        hr_bf = wk.tile([K, T], BF16, tag="hr_bf")
        for c in range(2):
            hrs = ps_acc.tile([K, 512], F32, tag="acc")
            nc.tensor.matmul(hrs[:], wt_hr[:], hr_in[:, c * 512:(c + 1) * 512])
            nc.scalar.activation(hr_bf[:, c * 512:(c + 1) * 512], hrs[:], AF.Tanh)

        # logits in transposed layout via 1-moving-row matmuls
        lg_ps = ps_acc.tile([128, 12], F32, tag="acc")
        for j in range(NI):
            nc.tensor.matmul(
                lg_ps[:, j:j + 1], hp_bf[:, j * 128:(j + 1) * 128], whp_b[:],
                skip_group_check=True,
            )
        for i in range(TI):
            nc.tensor.matmul(
                lg_ps[:, 4 + i:5 + i], hr_bf[:, i * 128:(i + 1) * 128], whr_b[:],
                skip_group_check=True,
            )
        nc.vector.tensor_copy(lgt_all[:, :, b], lg_ps[:])

    # ---------------- softmax phase (all batches on partitions) ----------------
    logits = inpool.tile([B_LOC, 12 * 128], F32)
    for g in range(3):
        lgt_t_ps = ps_acc.tile([B_LOC, 512], F32, tag="acc")
        for jj in range(4):
            j = g * 4 + jj
            nc.tensor.transpose(
                lgt_t_ps[:, jj * 128:(jj + 1) * 128], lgt_all[:, j, :], ident_f[:]
            )
        nc.vector.tensor_copy(logits[:, g * 512:(g + 1) * 512], lgt_t_ps[:])

    mx = inpool.tile([B_LOC, 2], F32)
    nc.vector.reduce_max(mx[:, 0:1], logits[:, 0:N], axis=mybir.AxisListType.X)
    nc.vector.reduce_max(mx[:, 1:2], logits[:, N:N + T], axis=mybir.AxisListType.X)
    nmx = inpool.tile([B_LOC, 2], F32)
    nc.vector.tensor_scalar_mul(nmx[:], mx[:], -1.0)

    probs = inpool.tile([B_LOC, 12 * 128], F32)
    sums = inpool.tile([B_LOC, 2], F32)
    nc.scalar.activation(
        probs[:, 0:N], logits[:, 0:N], AF.Exp, bias=nmx[:, 0:1], accum_out=sums[:, 0:1]
    )
    nc.scalar.activation(
        probs[:, N:N + T], logits[:, N:N + T], AF.Exp, bias=nmx[:, 1:2],
        accum_out=sums[:, 1:2],
    )
    rcp = inpool.tile([B_LOC, 2], F32)
    nc.vector.reciprocal(rcp[:], sums[:])
    pn = inpool.tile([B_LOC, 12 * 128], F32)
    nc.vector.tensor_scalar_mul(pn[:, 0:N], probs[:, 0:N], rcp[:, 0:1])
    nc.vector.tensor_scalar_mul(pn[:, N:N + T], probs[:, N:N + T], rcp[:, 1:2])

    # Transpose probs back to partition-major bf16: PrT[:, j, b]
    prt = inpool.tile([128, 12, B_LOC], BF16)
    prt_ps = ps_acc.tile([128, 12 * B_LOC], F32, tag="acc")
    for j in range(12):
        nc.tensor.transpose(
            prt_ps[:, j * B_LOC:(j + 1) * B_LOC],
            pn[:, j * 128:(j + 1) * 128],
            ident_f[0:B_LOC, 0:B_LOC],
        )
    nc.vector.tensor_copy(prt[:], prt_ps[:])

    # ---------------- pooling phase ----------------
    # co_all (64, 16): col b = P_b^T @ Ap_b, col 8+b = R_b^T @ Ar_b
    co_ps = ps_acc.tile([D, 2 * B_LOC], F32, tag="acc")
    for b in range(B_LOC):
        for j in range(NI):
            nc.tensor.matmul(
                co_ps[:, b:b + 1], p_ball[:, b, j], prt[:, j, b:b + 1],
                start=(j == 0), stop=(j == NI - 1), skip_group_check=True,
            )
        for i in range(TI):
            nc.tensor.matmul(
                co_ps[:, B_LOC + b:B_LOC + b + 1], r_ball[:, b, i],
                prt[:, 4 + i, b:b + 1],
                start=(i == 0), stop=(i == TI - 1), skip_group_check=True,
            )
    co_sb = inpool.tile([D, 2 * B_LOC], F32)
    nc.vector.tensor_copy(co_sb[:], co_ps[:])

    # Transpose (64, 16) -> (16, 64); row h*8+b is the h-half of out[b]
    cot_ps = ps_acc.tile([2 * B_LOC, D], F32, tag="acc")
    nc.tensor.transpose(cot_ps[:], co_sb[:], ident_f[0:D, 0:D])
    out_sb = inpool.tile([2 * B_LOC, D], F32)
    nc.vector.tensor_copy(out_sb[:], cot_ps[:])
    nc.sync.dma_start(out=out_e.ap()[:, 0:D], in_=out_sb[0:B_LOC, :])
    nc.sync.dma_start(out=out_e.ap()[:, D:2 * D], in_=out_sb[B_LOC:2 * B_LOC, :])
    ctx.close()


_NC_CACHE = None


def _get_nc():
    global _NC_CACHE
    if _NC_CACHE is None:
        _NC_CACHE = build_kernel()
    return _NC_CACHE


def _prep_host_inputs(inputs):
    import ml_dtypes

    bf = ml_dtypes.bfloat16
    rev = np.ascontiguousarray(np.asarray(inputs["review_seq"], dtype=np.float32))
    post = np.ascontiguousarray(np.asarray(inputs["post_seq"], dtype=np.float32))
    wl = np.asarray(inputs["Wl"], dtype=np.float32)
    wr = np.asarray(inputs["Wr"], dtype=np.float32)
    wp = np.asarray(inputs["Wp"], dtype=np.float32)
    whr = np.asarray(inputs["whr"], dtype=np.float32)
    whp = np.asarray(inputs["whp"], dtype=np.float32)

    rev_bf = rev.astype(bf)
    post_bf = post.astype(bf)
    rev_t = np.ascontiguousarray(np.swapaxes(rev_bf, 1, 2))
    post_t = np.ascontiguousarray(np.swapaxes(post_bf, 1, 2))
    const = {
        "wl_b": np.ascontiguousarray(wl.astype(bf)),
        "wt_hp": np.ascontiguousarray(np.concatenate([wp.T, wr.T], axis=0).astype(bf)),
        "wt_hr": np.ascontiguousarray(np.concatenate([wr.T, wp.T], axis=0).astype(bf)),
        "whp_c": np.ascontiguousarray(whp.T.astype(bf)),
        "whr_c": np.ascontiguousarray(whr.T.astype(bf)),
        "ident": np.eye(128, dtype=np.float32),
    }
    return rev_bf, rev_t, post_bf, post_t, const


def run_on_hw(inputs: dict, trace: bool = False, **kw):
    nc = _get_nc()
    rev_bf, rev_t, post_bf, post_t, const = _prep_host_inputs(inputs)
    in_maps = []
    for c in range(NCORES):
        s = slice(c * B_LOC, (c + 1) * B_LOC)
        m = {
            "review_bf": np.ascontiguousarray(rev_bf[s]),
            "review_t": np.ascontiguousarray(rev_t[s]),
            "post_bf": np.ascontiguousarray(post_bf[s]),
            "post_t": np.ascontiguousarray(post_t[s]),
        }
        m.update(const)
        in_maps.append(m)
    res = run_bass_kernel_spmd(nc, in_maps, list(range(NCORES)), trace=trace, **kw)
    out = np.concatenate([res.results[c]["out"] for c in range(NCORES)], axis=0)
    return out, res


def kernel(**inputs) -> np.ndarray:
    out, _ = run_on_hw(inputs, trace=False)
    return out.astype(np.float32)
